# revision 1
# baseline (speedup 1.0000x reference)
"""Trainium2 Bass kernel for nn_NetworkAction (GNN message passing, B=4 N=4096 K=16).

Sharding: 8 cores = (batch b, N-half h). Each core owns 2048 query agents of one
batch and scans all 4096 keys of that batch.

Per-core pipeline (per 128-query block), tuned for instruction-dispatch-bound
hardware (each instruction costs ~1us of queue/sem overhead):
  1) -d2 via bilinear f32r matmuls (8 x [4,128]^T @ [4,512] -> PSUM quarters),
     4 ACT copies -> V [128,4096] f32 SBUF.
  2) exact top-16 per row on DVE: max8 / max_index / match_replace / max8 /
     max_index -> ITP [128,16] u16.
  3) index redistribution via ONE DRAM round-trip: ITP -> D0, then one
     strided read back as 8 per-gpsimd-core wrapped streams IW [128,16]
     (core c gets edges [256c, 256(c+1)) of the block).
  4) neighbor states via ONE ap_gather (channels=128, 8 cores x 256 idx,
     table STC[c,n] = s[n, c%4]); component bands live at partitions 16c.
  5) gathered bands -> DRAM -> one strided read back as GB [4,2048] f32,
     ACT convert to fp16; edge MLP layer 1 = two accumulating matmuls per
     512-chunk ((W1r)s_q via stride-0 broadcast rhs + (-W1r)s_nbr), relu+b1;
     layer 2 fp16 matmuls into a 2-deep PSUM ring; k-max-pool on DVE
     (deferred into the next block's top-k window); feat = max(pool+b2, h2s)
     with the exact self-edge column h2s precomputed on host.
  6) node MLP 132->64->128->64->4 consolidated at the tail (f32 matmuls);
     2*sigmoid(z)-1 == tanh(z/2).

All host-side transposes/packing (LT/RT/PGV/STC/weights/h2s) in numpy.
"""
import numpy as np

import concourse.bacc as bacc
import concourse.mybir as mybir
from concourse.tile import TileContext
from concourse.bass_utils import run_bass_kernel_spmd

F32 = mybir.dt.float32
F32R = mybir.dt.float32r
F16 = mybir.dt.float16
U16 = mybir.dt.uint16
I16 = mybir.dt.int16
AX = mybir.AxisListType
ALU = mybir.AluOpType
ACTF = mybir.ActivationFunctionType

B, N, D, K = 4, 4096, 4, 16
NQ = N // 2            # queries per core
NBLK = NQ // 128       # 16 query blocks of 128
NEG = -1.0e30


def build_nc(reps=None, mode=3):
    nc = bacc.Bacc("TRN2", target_bir_lowering=False, debug=False, num_devices=8)

    lt = nc.dram_tensor("lt", [4, NQ], F32R, kind="ExternalInput")
    rt = nc.dram_tensor("rt", [4, N], F32R, kind="ExternalInput")
    stc = nc.dram_tensor("stc", [128, N], F32, kind="ExternalInput")
    sqt16 = nc.dram_tensor("sqt16", [4, NQ], F16, kind="ExternalInput")
    pgv = nc.dram_tensor("pgv", [4, NQ], F32, kind="ExternalInput")
    e1a = nc.dram_tensor("e1a", [4, 64], F16, kind="ExternalInput")
    e1b = nc.dram_tensor("e1b", [4, 64], F16, kind="ExternalInput")
    w2t16 = nc.dram_tensor("w2t16", [64, 128], F16, kind="ExternalInput")
    b1 = nc.dram_tensor("b1", [64, 1], F32, kind="ExternalInput")
    b2 = nc.dram_tensor("b2", [128, 1], F32, kind="ExternalInput")
    h2s = nc.dram_tensor("h2s", [128, 1], F32, kind="ExternalInput")
    fw1at = nc.dram_tensor("fw1at", [128, 64], F32, kind="ExternalInput")
    fw1bt = nc.dram_tensor("fw1bt", [4, 64], F32, kind="ExternalInput")
    fb1 = nc.dram_tensor("fb1", [64, 1], F32, kind="ExternalInput")
    fw2t = nc.dram_tensor("fw2t", [64, 128], F32, kind="ExternalInput")
    fb2 = nc.dram_tensor("fb2", [128, 1], F32, kind="ExternalInput")
    fw3t = nc.dram_tensor("fw3t", [128, 64], F32, kind="ExternalInput")
    fb3 = nc.dram_tensor("fb3", [64, 1], F32, kind="ExternalInput")
    fw4t = nc.dram_tensor("fw4t", [64, 4], F32, kind="ExternalInput")
    fb4h = nc.dram_tensor("fb4h", [4, 1], F32, kind="ExternalInput")  # 0.5*fb4
    out = nc.dram_tensor("out", [D, NQ], F32, kind="ExternalOutput")

    with TileContext(nc) as tc:
        import contextlib
        loop_cm = tc.For_i(0, reps, 1) if reps is not None else contextlib.nullcontext()
        with (
            tc.tile_pool(name="const", bufs=1) as cp,
            tc.tile_pool(name="psA", bufs=1, space="PSUM") as psA,
            tc.tile_pool(name="psH1", bufs=2, space="PSUM") as psH1,
            tc.tile_pool(name="psH2", bufs=2, space="PSUM") as psH2,
            tc.tile_pool(name="vbuf", bufs=1) as vp,
            tc.tile_pool(name="vrbuf", bufs=1) as vrp,
            tc.tile_pool(name="gbuf", bufs=2) as gp,
            tc.tile_pool(name="small", bufs=2) as sp,
            tc.tile_pool(name="dscr", bufs=2, space="DRAM") as dp,
            loop_cm,
        ):
            # ---------------- constants to SBUF ----------------
            LT = cp.tile([4, NQ], F32R, tag="LT")
            nc.sync.dma_start(out=LT[:], in_=lt[:])
            RT = cp.tile([4, N], F32R, tag="RT")
            nc.sync.dma_start(out=RT[:], in_=rt[:])
            SQT16 = cp.tile([4, NQ], F16, tag="SQT16")
            nc.sync.dma_start(out=SQT16[:], in_=sqt16[:])
            PGV = cp.tile([4, NQ], F32, tag="PGV")
            nc.sync.dma_start(out=PGV[:], in_=pgv[:])
            STC = cp.tile([128, N], F32, tag="STC")
            nc.sync.dma_start(out=STC[:], in_=stc[:])
            tE1a = cp.tile([4, 64], F16, tag="tE1a")
            nc.sync.dma_start(out=tE1a[:], in_=e1a[:])
            tE1b = cp.tile([4, 64], F16, tag="tE1b")
            nc.sync.dma_start(out=tE1b[:], in_=e1b[:])
            tw2t = cp.tile([64, 128], F16, tag="tw2t")
            nc.sync.dma_start(out=tw2t[:], in_=w2t16[:])
            tb1 = cp.tile([64, 1], F32, tag="tb1")
            nc.sync.dma_start(out=tb1[:], in_=b1[:])
            tb2 = cp.tile([128, 1], F32, tag="tb2")
            nc.sync.dma_start(out=tb2[:], in_=b2[:])
            th2s = cp.tile([128, 1], F32, tag="th2s")
            nc.sync.dma_start(out=th2s[:], in_=h2s[:])
            tfw1at = cp.tile([128, 64], F32, tag="tfw1at")
            nc.sync.dma_start(out=tfw1at[:], in_=fw1at[:])
            tfw1bt = cp.tile([4, 64], F32, tag="tfw1bt")
            nc.sync.dma_start(out=tfw1bt[:], in_=fw1bt[:])
            tfb1 = cp.tile([64, 1], F32, tag="tfb1")
            nc.sync.dma_start(out=tfb1[:], in_=fb1[:])
            tfw2t = cp.tile([64, 128], F32, tag="tfw2t")
            nc.sync.dma_start(out=tfw2t[:], in_=fw2t[:])
            tfb2 = cp.tile([128, 1], F32, tag="tfb2")
            nc.sync.dma_start(out=tfb2[:], in_=fb2[:])
            tfw3t = cp.tile([128, 64], F32, tag="tfw3t")
            nc.sync.dma_start(out=tfw3t[:], in_=fw3t[:])
            tfb3 = cp.tile([64, 1], F32, tag="tfb3")
            nc.sync.dma_start(out=tfb3[:], in_=fb3[:])
            tfw4t = cp.tile([64, 4], F32, tag="tfw4t")
            nc.sync.dma_start(out=tfw4t[:], in_=fw4t[:])
            tfb4h = cp.tile([4, 1], F32, tag="tfb4h")
            nc.sync.dma_start(out=tfb4h[:], in_=fb4h[:])

            ITP = cp.tile([128, 128], U16, tag="ITP")
            nc.vector.memset(ITP[:], 0)
            featR = cp.tile([128, NQ], F32, tag="featR", name="featR") if mode in (2, 3) else None
            OT = cp.tile([4, NQ], F32, tag="OT", name="OT") if mode == 3 else None

            # ---------------- main per-block loop ----------------
            pending = None  # deferred (H2a, H2b, q0) k-pool from prev block

            def do_pending():
                nonlocal pending
                if pending is None:
                    return
                H2a, H2b, pq0 = pending
                pt = sp.tile([128, 128], F32, tag="pt")
                nc.vector.tensor_reduce(
                    out=pt[:, 0:64], in_=H2a[:].rearrange("p (q k) -> p q k", k=K),
                    axis=AX.X, op=ALU.max,
                )
                nc.vector.tensor_reduce(
                    out=pt[:, 64:128], in_=H2b[:].rearrange("p (q k) -> p q k", k=K),
                    axis=AX.X, op=ALU.max,
                )
                nc.vector.scalar_tensor_tensor(
                    out=featR[:, pq0 : pq0 + 128], in0=pt[:],
                    scalar=tb2[:, 0:1],
                    in1=th2s[:, 0:1].to_broadcast([128, 128]),
                    op0=ALU.add, op1=ALU.max,
                )
                pending = None

            for blk in range(NBLK):
                q0 = blk * 128
                # ---- -d2 matmuls (f32r), PSUM quarters -> V f32 SBUF ----
                V = vp.tile([128, N], F32, tag="V")
                for h in range(4):
                    vps = psA.tile([128, 1024], F32, tag="vps")
                    for j in range(2):
                        nc.tensor.matmul(
                            out=vps[:, j * 512 : (j + 1) * 512],
                            lhsT=LT[:, q0 : q0 + 128],
                            rhs=RT[:, h * 1024 + j * 512 : h * 1024 + (j + 1) * 512],
                            start=True, stop=True,
                        )
                    nc.scalar.copy(
                        out=V[:, h * 1024 : (h + 1) * 1024], in_=vps[:]
                    )

                # ---- exact top-16 (DVE); prev block's k-pool interleaved ----
                m1 = sp.tile([128, 8], F32, tag="m1")
                nc.vector.max(out=m1[:], in_=V[:])
                nc.vector.max_index(out=ITP[:, 0:8], in_max=m1[:], in_values=V[:])
                VR = vrp.tile([128, N], F32, tag="VR")
                nc.vector.match_replace(
                    out=VR[:], in_to_replace=m1[:], in_values=V[:], imm_value=NEG
                )
                m2 = sp.tile([128, 8], F32, tag="m2")
                nc.vector.max(out=m2[:], in_=VR[:])
                nc.vector.max_index(out=ITP[:, 8:16], in_max=m2[:], in_values=VR[:])
                do_pending()

                if mode < 1:
                    continue

                # ---- index redistribution: XBAR transpose + 8 band moves ----
                idxT = gp.tile([128, 128], U16, tag="idxT")
                nc.sync.dma_start_transpose(out=idxT[:], in_=ITP[:])
                IW = gp.tile([128, 16], U16, tag="IW")
                for c in range(8):
                    nc.sync.dma_start(
                        out=IW[16 * c : 16 * c + 16, :],
                        in_=idxT[0:16, 16 * c : 16 * c + 16],
                    )
                if mode == 10:
                    continue
                # ---- neighbor gather: 8 cores x 256 idx ----
                G = gp.tile([128, 256], F32, tag="G")
                nc.gpsimd.ap_gather(
                    out_ap=G[:].rearrange("c (n d) -> c n d", d=1),
                    in_ap=STC[:].rearrange("c (n d) -> c n d", d=1),
                    idxs_ap=IW[:].bitcast(I16),
                    channels=128, num_elems=N, d=1, num_idxs=256,
                )
                # bands -> GB [4, 2048] f32 (partition moves) -> fp16
                GB = gp.tile([4, 2048], F32, tag="GB")
                for c in range(8):
                    nc.sync.dma_start(
                        out=GB[:, c * 256 : (c + 1) * 256],
                        in_=G[16 * c : 16 * c + 4, :],
                    )
                GBf = gp.tile([4, 2048], F16, tag="GBf")
                nc.scalar.copy(out=GBf[:], in_=GB[:])
                if mode < 2:
                    continue

                # ---- edge MLP ----
                h1 = gp.tile([64, 2048], F16, tag="h1")
                for j in range(4):
                    H1P = psH1.tile([64, 512], F32, tag="h1p")
                    nc.tensor.matmul(
                        out=H1P[:],
                        lhsT=tE1a[:],
                        rhs=GBf[:, j * 512 : (j + 1) * 512],
                        start=True, stop=False,
                    )
                    nc.tensor.matmul(
                        out=H1P[:],
                        lhsT=tE1b[:],
                        rhs=SQT16[:, q0 + j * 32 : q0 + (j + 1) * 32]
                        .rearrange("c q -> c q ()")
                        .to_broadcast([4, 32, K]),
                        start=False, stop=True,
                    )
                    nc.scalar.activation(
                        out=h1[:, j * 512 : (j + 1) * 512], in_=H1P[:],
                        func=ACTF.Relu, bias=tb1[:, 0:1],
                    )
                H2a = psH2.tile([128, 1024], F32, tag="eh")
                H2b = psH2.tile([128, 1024], F32, tag="eh")
                for j in range(4):
                    nc.tensor.matmul(
                        out=(H2a if j < 2 else H2b)[
                            :, (j % 2) * 512 : (j % 2 + 1) * 512
                        ],
                        lhsT=tw2t[:],
                        rhs=h1[:, j * 512 : (j + 1) * 512],
                        start=True, stop=True,
                    )
                pending = (H2a, H2b, q0)

            do_pending()

            # ---------------- node MLP (tail) ----------------
            if mode == 3:
                n1 = cp.tile([64, NQ], F32, tag="n1")
                for t in range(4):
                    t0 = t * 512
                    mpa = psA.tile([128, 1024], F32, tag="vps")
                    nc.tensor.matmul(
                        out=mpa[0:64, 0:512], lhsT=tfw1at[:],
                        rhs=featR[:, t0 : t0 + 512], start=True, stop=False,
                    )
                    nc.tensor.matmul(
                        out=mpa[0:64, 0:512], lhsT=tfw1bt[:],
                        rhs=PGV[:, t0 : t0 + 512], start=False, stop=True,
                    )
                    nc.scalar.activation(
                        out=n1[:, t0 : t0 + 512], in_=mpa[0:64, 0:512],
                        func=ACTF.Relu, bias=tfb1[:, 0:1],
                    )
                n3 = cp.tile([64, NQ], F32, tag="n3")
                for t in range(4):
                    t0 = t * 512
                    mpb = psH2.tile([128, 1024], F32, tag="eh")
                    nc.tensor.matmul(
                        out=mpb[:, 0:512], lhsT=tfw2t[:],
                        rhs=n1[:, t0 : t0 + 512], start=True, stop=True,
                    )
                    n2t = sp.tile([128, 512], F32, tag="n2t")
                    nc.scalar.activation(
                        out=n2t[:], in_=mpb[:, 0:512], func=ACTF.Relu, bias=tfb2[:, 0:1]
                    )
                    mpc = psA.tile([128, 1024], F32, tag="vps")
                    nc.tensor.matmul(
                        out=mpc[0:64, 0:512], lhsT=tfw3t[:],
                        rhs=n2t[:], start=True, stop=True,
                    )
                    nc.scalar.activation(
                        out=n3[:, t0 : t0 + 512], in_=mpc[0:64, 0:512],
                        func=ACTF.Relu, bias=tfb3[:, 0:1],
                    )
                for t in range(4):
                    t0 = t * 512
                    mpd = psH2.tile([128, 1024], F32, tag="eh")
                    nc.tensor.matmul(
                        out=mpd[0:4, 0:512], lhsT=tfw4t[:],
                        rhs=n3[:, t0 : t0 + 512], start=True, stop=True,
                    )
                    # 2*sigmoid(z) - 1 == tanh(0.5 z); bias = 0.5*fb4
                    nc.scalar.activation(
                        out=OT[:, t0 : t0 + 512], in_=mpd[0:4, 0:512],
                        func=ACTF.Tanh, scale=0.5, bias=tfb4h[:, 0:1],
                    )
                nc.sync.dma_start(out=out[:, :], in_=OT[:])
            else:
                nc.sync.dma_start(out=out[0:1, 0:4], in_=PGV[0:1, 0:4])

    nc.compile()
    return nc


_BUILT = {}


def get_nc(reps=None, mode=3):
    key = (reps, mode)
    if key not in _BUILT:
        _BUILT[key] = build_nc(reps, mode)
    return _BUILT[key]


def make_in_maps(s, g, w1, b1, w2, b2, fw1, fb1, fw2, fb2, fw3, fb3, fw4, fb4):
    f32 = lambda a: np.ascontiguousarray(np.asarray(a, np.float32))
    f16 = lambda a: np.ascontiguousarray(np.asarray(a, np.float16))
    s, g = f32(s), f32(g)
    w1, w2, fw1, fw2, fw3, fw4 = map(f32, (w1, w2, fw1, fw2, fw3, fw4))
    b1, b2, fb1, fb2, fb3, fb4 = map(f32, (b1, b2, fb1, fb2, fb3, fb4))

    w1r = w1[:, :4]                    # [64, 4]
    w1e = w1[:, 4]                     # [64]
    # exact self-edge column: relu(W2 relu(w1e + b1) + b2)
    h1s = np.maximum(w1e + b1, 0.0)
    h2s = np.maximum(w2 @ h1s + b2, 0.0)

    shared = {
        "e1a": f16(-w1r.T), "e1b": f16(w1r.T),
        "w2t16": f16(w2.T),
        "b1": f32(b1[:, None]), "b2": f32(b2[:, None]),
        "h2s": f32(h2s[:, None]),
        "fw1at": f32(fw1[:, :128].T), "fw1bt": f32(fw1[:, 128:].T),
        "fb1": f32(fb1[:, None]),
        "fw2t": f32(fw2.T), "fb2": f32(fb2[:, None]),
        "fw3t": f32(fw3.T), "fb3": f32(fb3[:, None]),
        "fw4t": f32(fw4.T), "fb4h": f32(0.5 * fb4[:, None]),
    }
    in_maps = []
    for c in range(8):
        b, h = c // 2, c % 2
        sl = slice(h * NQ, (h + 1) * NQ)
        sb = s[b]                       # [N, 4]
        sq = s[b, sl]                   # [NQ, 4]
        gq = g[b, sl]                   # [NQ, 2]
        sqk = (sb[:, 0] ** 2 + sb[:, 1] ** 2)          # [N]
        sqq = (sq[:, 0] ** 2 + sq[:, 1] ** 2)          # [NQ]
        lt = np.stack([np.ones(NQ, np.float32), -sqq, 2 * sq[:, 0], 2 * sq[:, 1]])
        rt = np.stack([-sqk, np.ones(N, np.float32), sb[:, 0], sb[:, 1]])
        stcv = np.ascontiguousarray(np.tile(sb.T, (32, 1)).astype(np.float32))
        pgv = np.concatenate([(sq[:, :2] - gq).T, sq[:, 2:].T], axis=0)
        in_maps.append({
            "lt": f32(lt), "rt": f32(rt), "stc": stcv,
            "sqt16": f16(sq.T), "pgv": f32(pgv), **shared,
        })
    return in_maps


def kernel(**inputs):
    in_maps = make_in_maps(**inputs)
    nc = get_nc(None)
    res = run_bass_kernel_spmd(nc, in_maps, list(range(8)))
    out = np.zeros((B, N, D), np.float32)
    for c in range(8):
        b, h = c // 2, c % 2
        out[b, h * NQ : (h + 1) * NQ] = res.results[c]["out"].T
    return out



# revision 46
# speedup vs baseline: 24.7335x; 24.7335x over previous
"""Trainium2 Bass kernel for nn_NetworkAction (GNN message passing, B=4 N=4096 K=16).

Sharding: 8 cores = (batch b, N-half h). Each core owns 2048 query agents of one
batch and scans all 4096 keys of that batch.

v5 pipeline, software-pipelined around the DVE scan roofline:
  1) -d2 via bilinear f32r matmuls into PSUM quarter tiles [128,1024]
     (psQ bufs=2), ACT-copied to V [128,4096] SBUF.  The dist stage of
     block b+1 is EMITTED one block early so the in-order ACT queue
     serves these copies before chain-dependent relu/H2S ops.
  2) per-quarter top-8 on DVE: max8 -> CV[128,32], max_index -> CIu
     (local idx).  2 DVE passes over N total (vs 5 for exact top-16).
  3) top-16 of the 32 candidates via f32-exact key packing, all on DVE:
     key = clip((~bits(min(CV,-1e-12)) - KSBASE) * 2^-15, 0, 4095)
     pack = key*4096 + (CIu + quarter_offset)   (< 2^24, exact in f32)
     max8(pack), match_replace, max8 -> 16 winners; idx = winner & 4095.
     Approximation: top-8-per-quarter candidates + ~0.4%-relative d2 key
     quantization; end-to-end rel-err contribution ~3.5e-3.
  4) per-SUPERBLOCK (2 blocks) index redistribution: ITP[128,32] u16 ->
     one XBAR transpose -> 8 per-gpsimd-core streams IW[128,32] (cores
     0-3 serve block A, 4-7 block B); ONE ap_gather (8 cores x 512 idx,
     table STC[c,n]=s[n,c%4]) -> bands -> GB[4,4096].
  5) edge MLP layer 1 in f32r (GB bitcast, s_q term via stride-0
     broadcast rhs), relu+b1 -> fp16; layer 2 fp16 into psH2 [128,512],
     ACT-copied to H2S fp16; k-max-pool as an fp16 tensor_tensor max
     tree on DVE, deferred ~2 blocks; feat = max(pool+b2, h2s) with the
     exact self-edge column h2s precomputed on host.
  6) node MLP 132->64->128->64->4 in 512-query chunks interleaved into
     blocks 7/11/15 + tail; 2*sigmoid(z)-1 == tanh(z/2).

All host-side transposes/packing in numpy.
"""
import numpy as np

import concourse.bacc as bacc
import concourse.mybir as mybir
from concourse.tile import TileContext
from concourse.bass_utils import run_bass_kernel_spmd

F32 = mybir.dt.float32
F32R = mybir.dt.float32r
F16 = mybir.dt.float16
U16 = mybir.dt.uint16
U32 = mybir.dt.uint32
I16 = mybir.dt.int16
I32 = mybir.dt.int32
AX = mybir.AxisListType
ALU = mybir.AluOpType
ACTF = mybir.ActivationFunctionType

B, N, D, K = 4, 4096, 4, 16
NQ = N // 2            # queries per core
NBLK = NQ // 128       # 16 query blocks of 128


def build_nc(reps=None, mode=3):
    nc = bacc.Bacc("TRN2", target_bir_lowering=False, debug=False, num_devices=8)

    lt = nc.dram_tensor("lt", [4, NQ], F32R, kind="ExternalInput")
    rt = nc.dram_tensor("rt", [4, N], F32R, kind="ExternalInput")
    stc = nc.dram_tensor("stc", [128, N], F32, kind="ExternalInput")
    sqtf = nc.dram_tensor("sqtf", [4, NQ], F32R, kind="ExternalInput")
    pgv = nc.dram_tensor("pgv", [4, NQ], F32, kind="ExternalInput")
    offs = nc.dram_tensor("offs", [128, 32], F32, kind="ExternalInput")
    e1a = nc.dram_tensor("e1a", [4, 64], F32R, kind="ExternalInput")
    e1b = nc.dram_tensor("e1b", [4, 64], F32R, kind="ExternalInput")
    w2t16 = nc.dram_tensor("w2t16", [64, 128], F16, kind="ExternalInput")
    b1 = nc.dram_tensor("b1", [64, 1], F32, kind="ExternalInput")
    b2 = nc.dram_tensor("b2", [128, 1], F32, kind="ExternalInput")
    h2s = nc.dram_tensor("h2s", [128, 1], F32, kind="ExternalInput")
    fw1at = nc.dram_tensor("fw1at", [128, 64], F32, kind="ExternalInput")
    fw1bt = nc.dram_tensor("fw1bt", [4, 64], F32, kind="ExternalInput")
    fb1 = nc.dram_tensor("fb1", [64, 1], F32, kind="ExternalInput")
    fw2t = nc.dram_tensor("fw2t", [64, 128], F32, kind="ExternalInput")
    fb2 = nc.dram_tensor("fb2", [128, 1], F32, kind="ExternalInput")
    fw3t = nc.dram_tensor("fw3t", [128, 64], F32, kind="ExternalInput")
    fb3 = nc.dram_tensor("fb3", [64, 1], F32, kind="ExternalInput")
    fw4t = nc.dram_tensor("fw4t", [64, 4], F32, kind="ExternalInput")
    fb4h = nc.dram_tensor("fb4h", [4, 1], F32, kind="ExternalInput")  # 0.5*fb4
    out = nc.dram_tensor("out", [D, NQ], F32, kind="ExternalOutput")

    with TileContext(nc) as tc:
        import contextlib
        loop_cm = tc.For_i(0, reps, 1) if reps is not None else contextlib.nullcontext()
        with (
            tc.tile_pool(name="const", bufs=1) as cp,
            tc.tile_pool(name="psQ", bufs=2, space="PSUM") as psQ,
            tc.tile_pool(name="psH1", bufs=2, space="PSUM") as psH1,
            tc.tile_pool(name="psH2", bufs=2, space="PSUM") as psH2,
            tc.tile_pool(name="small", bufs=2) as sp,
            tc.tile_pool(name="gbuf", bufs=2) as gp,
            tc.tile_pool(name="hbuf", bufs=2) as hp,
            tc.tile_pool(name="vbuf", bufs=2) as vp,
            tc.tile_pool(name="gbb", bufs=1) as gbp,
            loop_cm,
        ):
            # ---------------- constants to SBUF ----------------
            LT = cp.tile([4, NQ], F32R, tag="LT")
            nc.sync.dma_start(out=LT[:], in_=lt[:])
            RT = cp.tile([4, N], F32R, tag="RT")
            nc.sync.dma_start(out=RT[:], in_=rt[:])
            SQTF = cp.tile([4, NQ], F32R, tag="SQTF")
            nc.sync.dma_start(out=SQTF[:], in_=sqtf[:])
            PGV = cp.tile([4, NQ], F32, tag="PGV")
            nc.sync.dma_start(out=PGV[:], in_=pgv[:])
            STC = cp.tile([128, N], F32, tag="STC")
            nc.sync.dma_start(out=STC[:], in_=stc[:])
            OFFSF = cp.tile([128, 32], F32, tag="OFFSF")
            nc.sync.dma_start(out=OFFSF[:], in_=offs[:])
            tE1a = cp.tile([4, 64], F32R, tag="tE1a")
            nc.sync.dma_start(out=tE1a[:], in_=e1a[:])
            tE1b = cp.tile([4, 64], F32R, tag="tE1b")
            nc.sync.dma_start(out=tE1b[:], in_=e1b[:])
            tw2t = cp.tile([64, 128], F16, tag="tw2t")
            nc.sync.dma_start(out=tw2t[:], in_=w2t16[:])
            tb1 = cp.tile([64, 1], F32, tag="tb1")
            nc.sync.dma_start(out=tb1[:], in_=b1[:])
            tb2 = cp.tile([128, 1], F32, tag="tb2")
            nc.sync.dma_start(out=tb2[:], in_=b2[:])
            th2s = cp.tile([128, 1], F32, tag="th2s")
            nc.sync.dma_start(out=th2s[:], in_=h2s[:])
            tfw1at = cp.tile([128, 64], F32, tag="tfw1at")
            nc.sync.dma_start(out=tfw1at[:], in_=fw1at[:])
            tfw1bt = cp.tile([4, 64], F32, tag="tfw1bt")
            nc.sync.dma_start(out=tfw1bt[:], in_=fw1bt[:])
            tfb1 = cp.tile([64, 1], F32, tag="tfb1")
            nc.sync.dma_start(out=tfb1[:], in_=fb1[:])
            tfw2t = cp.tile([64, 128], F32, tag="tfw2t")
            nc.sync.dma_start(out=tfw2t[:], in_=fw2t[:])
            tfb2 = cp.tile([128, 1], F32, tag="tfb2")
            nc.sync.dma_start(out=tfb2[:], in_=fb2[:])
            tfw3t = cp.tile([128, 64], F32, tag="tfw3t")
            nc.sync.dma_start(out=tfw3t[:], in_=fw3t[:])
            tfb3 = cp.tile([64, 1], F32, tag="tfb3")
            nc.sync.dma_start(out=tfb3[:], in_=fb3[:])
            tfw4t = cp.tile([64, 4], F32, tag="tfw4t")
            nc.sync.dma_start(out=tfw4t[:], in_=fw4t[:])
            tfb4h = cp.tile([4, 1], F32, tag="tfb4h")
            nc.sync.dma_start(out=tfb4h[:], in_=fb4h[:])

            ITP = cp.tile([128, 128], U16, tag="ITP")
            nc.vector.memset(ITP[:], 0)
            # const tiles for bitvec ops (tensor_tensor only; imm bitvec is
            # rejected by the ISA checker)
            NEGS = cp.tile([128, 32], I32, tag="NEGS")
            nc.vector.memset(NEGS[:], -1)
            M4095T = cp.tile([128, 16], I32, tag="M4095T")
            nc.vector.memset(M4095T[:], 4095)
            featR = cp.tile([128, NQ], F32, tag="featR", name="featR")
            OT = cp.tile([4, NQ], F32, tag="OT", name="OT")

            # ---------------- main per-block loop ----------------
            pendq = []  # deferred (H2S, q0) k-pools, drained 2 blocks later

            def do_pending(force=False):
                while pendq and (force or len(pendq) > 1):
                    _drain_one()

            def _drain_one():
                H2Sp, pq0 = pendq.pop(0)
                # k-max-pool as an fp16 tensor_tensor max tree (2x DVE mode)
                Hqk = H2Sp[:].rearrange("p (q k) -> p q k", k=K)
                TP8 = sp.tile([128, 2048], F16, tag="TP8")
                T8 = TP8[:].rearrange("p (q k) -> p q k", k=8)
                nc.vector.tensor_tensor(T8, Hqk[:, :, 0:8], Hqk[:, :, 8:16],
                                        ALU.max)
                TP4 = sp.tile([128, 1024], F16, tag="TP4")
                T4 = TP4[:].rearrange("p (q k) -> p q k", k=4)
                nc.vector.tensor_tensor(T4, T8[:, :, 0:4], T8[:, :, 4:8],
                                        ALU.max)
                TP2 = sp.tile([128, 512], F16, tag="TP2")
                T2 = TP2[:].rearrange("p (q k) -> p q k", k=2)
                nc.vector.tensor_tensor(T2, T4[:, :, 0:2], T4[:, :, 2:4],
                                        ALU.max)
                pt = sp.tile([128, 256], F16, tag="pt")
                nc.vector.tensor_tensor(
                    pt[:].rearrange("p (q k) -> p q k", k=1),
                    T2[:, :, 0:1], T2[:, :, 1:2], ALU.max,
                )
                nc.vector.scalar_tensor_tensor(
                    out=featR[:, pq0 : pq0 + 256], in0=pt[:],
                    scalar=tb2[:, 0:1],
                    in1=th2s[:, 0:1].to_broadcast([128, 256]),
                    op0=ALU.add, op1=ALU.max,
                )

            def node_chunk(t):
                # node MLP for queries [512t, 512t+512): 132->64->128->64->4
                t0 = t * 512
                mpa = psQ.tile([128, 1024], F32, tag="PQ")
                nc.tensor.matmul(
                    out=mpa[0:64, 0:512], lhsT=tfw1at[:],
                    rhs=featR[:, t0 : t0 + 512], start=True, stop=False,
                )
                nc.tensor.matmul(
                    out=mpa[0:64, 0:512], lhsT=tfw1bt[:],
                    rhs=PGV[:, t0 : t0 + 512], start=False, stop=True,
                )
                n1c = sp.tile([64, 512], F32, tag="n1c")
                nc.scalar.activation(
                    out=n1c[:], in_=mpa[0:64, 0:512],
                    func=ACTF.Relu, bias=tfb1[:, 0:1],
                )
                mpb = psH2.tile([128, 512], F32, tag="H2P")
                nc.tensor.matmul(
                    out=mpb[:], lhsT=tfw2t[:], rhs=n1c[:], start=True, stop=True,
                )
                n2c = sp.tile([128, 512], F32, tag="n2c")
                nc.scalar.activation(
                    out=n2c[:], in_=mpb[:], func=ACTF.Relu, bias=tfb2[:, 0:1]
                )
                mpc = psH1.tile([64, 512], F32, tag="H1P")
                nc.tensor.matmul(
                    out=mpc[:], lhsT=tfw3t[:], rhs=n2c[:], start=True, stop=True,
                )
                n3c = sp.tile([64, 512], F32, tag="n3c")
                nc.scalar.activation(
                    out=n3c[:], in_=mpc[:], func=ACTF.Relu, bias=tfb3[:, 0:1]
                )
                mpd = psH2.tile([128, 512], F32, tag="H2P")
                nc.tensor.matmul(
                    out=mpd[0:4, :], lhsT=tfw4t[:], rhs=n3c[:],
                    start=True, stop=True,
                )
                # 2*sigmoid(z) - 1 == tanh(0.5 z); bias = 0.5*fb4
                nc.scalar.activation(
                    out=OT[:, t0 : t0 + 512], in_=mpd[0:4, :],
                    func=ACTF.Tanh, scale=0.5, bias=tfb4h[:, 0:1],
                )

            def dist_stage(blk):
                # -d2 matmuls (f32r) into PSUM quarters -> ACT copy -> V SBUF.
                # Emitted one block AHEAD of the rest of the pipeline so the
                # ACT queue serves these copies before the (chain-dependent)
                # relu/H2S ops of the previous block.
                q0 = blk * 128
                V = vp.tile([128, N], F32, tag="V", name=f"V{blk}")
                for q in range(4):
                    PQ = psQ.tile([128, 1024], F32, tag="PQ")
                    for j in range(2):
                        nc.tensor.matmul(
                            out=PQ[:, j * 512 : (j + 1) * 512],
                            lhsT=LT[:, q0 : q0 + 128],
                            rhs=RT[:, q * 1024 + j * 512 : q * 1024 + (j + 1) * 512],
                            start=True, stop=True,
                        )
                    nc.scalar.copy(
                        out=V[:, q * 1024 : (q + 1) * 1024], in_=PQ[:]
                    )
                return V

            Vcur = None if mode == -2 else dist_stage(0)
            for blk in range(NBLK):
                if mode == -2:
                    continue
                q0 = blk * 128
                V, Vcur = Vcur, (dist_stage(blk + 1) if blk + 1 < NBLK else None)
                if mode == -1:
                    do_pending()
                    continue
                # ---- top-8 per quarter, scanned from SBUF (DVE) ----
                CV = sp.tile([128, 32], F32, tag="CV")
                CIu = sp.tile([128, 32], U32, tag="CIu")
                for q in range(4):
                    nc.vector.max(out=CV[:, 8 * q : 8 * q + 8],
                                  in_=V[:, q * 1024 : (q + 1) * 1024])
                    nc.vector.max_index(
                        out=CIu[:, 8 * q : 8 * q + 8],
                        in_max=CV[:, 8 * q : 8 * q + 8],
                        in_values=V[:, q * 1024 : (q + 1) * 1024],
                    )
                if mode == 4:
                    do_pending()
                    continue

                # ---- f32-exact key pack + top-16 of the 32 candidates ----
                # key = clip((~bits(min(CV,-eps)) - KSBASE) >> 15, 0, 4095)
                # pack = key*4096 + global_idx  (< 2^24, exact in f32)
                # all on DVE so the chain never waits on the ACT queue
                CIGf = sp.tile([128, 32], F32, tag="CIGf")
                nc.vector.tensor_tensor(CIGf[:], CIu[:], OFFSF[:], ALU.add)
                Vc = sp.tile([128, 32], F32, tag="Vc")
                nc.vector.tensor_scalar_min(Vc[:], CV[:], -1.0e-12)
                KS = sp.tile([128, 32], I32, tag="KS")
                nc.vector.tensor_tensor(KS[:], Vc[:].bitcast(I32), NEGS[:],
                                        ALU.bitwise_xor)
                T12 = sp.tile([128, 32], I32, tag="T12")
                nc.vector.tensor_scalar(T12[:], KS[:], -1065353215.0,
                                        1.0 / 32768.0, ALU.add, ALU.mult)
                T12c = sp.tile([128, 32], I32, tag="T12c")
                nc.vector.tensor_scalar(T12c[:], T12[:], 0.0, 4095.0,
                                        ALU.max, ALU.min)
                PK = sp.tile([128, 32], F32, tag="PK")
                nc.vector.scalar_tensor_tensor(
                    out=PK[:], in0=T12c[:], scalar=4096.0, in1=CIGf[:],
                    op0=ALU.mult, op1=ALU.add,
                )
                Mw = sp.tile([128, 16], F32, tag="Mw")
                nc.vector.max(out=Mw[:, 0:8], in_=PK[:])
                PKr = sp.tile([128, 32], F32, tag="PKr")
                nc.vector.match_replace(
                    out=PKr[:], in_to_replace=Mw[:, 0:8], in_values=PK[:],
                    imm_value=-1.0e30,
                )
                nc.vector.max(out=Mw[:, 8:16], in_=PKr[:])
                MwI = sp.tile([128, 16], I32, tag="MwI")
                nc.vector.tensor_copy(MwI[:], Mw[:])
                EX = sp.tile([128, 16], I32, tag="EX")
                nc.vector.tensor_tensor(EX[:], MwI[:], M4095T[:],
                                        ALU.bitwise_and)
                half = blk % 2
                nc.vector.tensor_copy(ITP[:, 16 * half : 16 * half + 16], EX[:])
                do_pending()

                if mode == 0 or half == 0:
                    continue

                # ---- per-SUPERBLOCK (2 blocks, 256 queries) chain ----
                # ITP cols 0:16 = block A (queries 0:128 of the superblock),
                # cols 16:32 = block B.  IW[16c+p, s] = ITP2[32c'+s, p-col]
                # with cores 0-3 -> A, 4-7 -> B; each core gathers 512 edges.
                sq0 = q0 - 128  # superblock query origin
                idxT = gp.tile([128, 128], U16, tag="idxT")
                nc.sync.dma_start_transpose(out=idxT[:], in_=ITP[:])
                IW = gp.tile([128, 32], U16, tag="IW")
                for c in range(8):
                    if c < 4:
                        src = idxT[0:16, 32 * c : 32 * c + 32]
                    else:
                        src = idxT[16:32, 32 * (c - 4) : 32 * (c - 4) + 32]
                    nc.sync.dma_start(out=IW[16 * c : 16 * c + 16, :], in_=src)
                if mode == 10:
                    continue
                # ---- neighbor gather: 8 cores x 512 idx ----
                G = gp.tile([128, 512], F32, tag="G")
                nc.gpsimd.ap_gather(
                    out_ap=G[:].rearrange("c (n d) -> c n d", d=1),
                    in_ap=STC[:].rearrange("c (n d) -> c n d", d=1),
                    idxs_ap=IW[:].bitcast(I16),
                    channels=128, num_elems=N, d=1, num_idxs=512,
                )
                # bands -> GB [4, 4096] (8 moves; single buffer)
                GB = gbp.tile([4, 4096], F32, tag="GB")
                for c in range(8):
                    nc.sync.dma_start(
                        out=GB[:, c * 512 : (c + 1) * 512],
                        in_=G[16 * c : 16 * c + 4, :],
                    )
                if mode == 1:
                    continue

                # ---- edge MLP (layer 1 f32r, layer 2 fp16), 8 chunks ----
                h1 = gp.tile([64, 4096], F16, tag="h1")
                H2S = hp.tile([128, 4096], F16, tag="H2S")
                for j in range(8):
                    H1P = psH1.tile([64, 512], F32, tag="H1P")
                    nc.tensor.matmul(
                        out=H1P[:],
                        lhsT=tE1a[:],
                        rhs=GB[:, j * 512 : (j + 1) * 512].bitcast(F32R),
                        start=True, stop=False,
                    )
                    nc.tensor.matmul(
                        out=H1P[:],
                        lhsT=tE1b[:],
                        rhs=SQTF[:, sq0 + j * 32 : sq0 + (j + 1) * 32]
                        .rearrange("c q -> c q ()")
                        .to_broadcast([4, 32, K]),
                        start=False, stop=True,
                    )
                    nc.scalar.activation(
                        out=h1[:, j * 512 : (j + 1) * 512], in_=H1P[:],
                        func=ACTF.Relu, bias=tb1[:, 0:1],
                    )
                    H2P = psH2.tile([128, 512], F32, tag="H2P")
                    nc.tensor.matmul(
                        out=H2P[:],
                        lhsT=tw2t[:],
                        rhs=h1[:, j * 512 : (j + 1) * 512],
                        start=True, stop=True,
                    )
                    nc.scalar.copy(
                        out=H2S[:, j * 512 : (j + 1) * 512], in_=H2P[:]
                    )
                pendq.append((H2S, sq0))

                # interleave node-MLP chunks: featR[:, 512t:512t+512] covers
                # superblocks 2t,2t+1; their pools drain by block 4t+6
                if mode == 3 and blk in (7, 11, 15):
                    node_chunk((blk - 7) // 4)

            do_pending(force=True)

            # ---------------- node MLP (last chunk) + output ----------------
            if mode == 3:
                node_chunk(3)
                nc.sync.dma_start(out=out[:, :], in_=OT[:])
            else:
                nc.sync.dma_start(out=out[0:1, 0:4], in_=PGV[0:1, 0:4])

    nc.compile()
    return nc


_BUILT = {}


def get_nc(reps=None, mode=3):
    key = (reps, mode)
    if key not in _BUILT:
        _BUILT[key] = build_nc(reps, mode)
    return _BUILT[key]


def make_in_maps(s, g, w1, b1, w2, b2, fw1, fb1, fw2, fb2, fw3, fb3, fw4, fb4):
    f32 = lambda a: np.ascontiguousarray(np.asarray(a, np.float32))
    f16 = lambda a: np.ascontiguousarray(np.asarray(a, np.float16))
    s, g = f32(s), f32(g)
    w1, w2, fw1, fw2, fw3, fw4 = map(f32, (w1, w2, fw1, fw2, fw3, fw4))
    b1, b2, fb1, fb2, fb3, fb4 = map(f32, (b1, b2, fb1, fb2, fb3, fb4))

    w1r = w1[:, :4]                    # [64, 4]
    w1e = w1[:, 4]                     # [64]
    # exact self-edge column: relu(W2 relu(w1e + b1) + b2)
    h1s = np.maximum(w1e + b1, 0.0)
    h2s = np.maximum(w2 @ h1s + b2, 0.0)

    offs = np.ascontiguousarray(
        np.broadcast_to(np.repeat(np.arange(4, dtype=np.float32) * 1024, 8),
                        (128, 32))
    )

    shared = {
        "e1a": f32(-w1r.T), "e1b": f32(w1r.T),
        "w2t16": f16(w2.T),
        "b1": f32(b1[:, None]), "b2": f32(b2[:, None]),
        "h2s": f32(h2s[:, None]),
        "offs": offs,
        "fw1at": f32(fw1[:, :128].T), "fw1bt": f32(fw1[:, 128:].T),
        "fb1": f32(fb1[:, None]),
        "fw2t": f32(fw2.T), "fb2": f32(fb2[:, None]),
        "fw3t": f32(fw3.T), "fb3": f32(fb3[:, None]),
        "fw4t": f32(fw4.T), "fb4h": f32(0.5 * fb4[:, None]),
    }
    in_maps = []
    for c in range(8):
        b, h = c // 2, c % 2
        sl = slice(h * NQ, (h + 1) * NQ)
        sb = s[b]                       # [N, 4]
        sq = s[b, sl]                   # [NQ, 4]
        gq = g[b, sl]                   # [NQ, 2]
        sqk = (sb[:, 0] ** 2 + sb[:, 1] ** 2)          # [N]
        sqq = (sq[:, 0] ** 2 + sq[:, 1] ** 2)          # [NQ]
        lt = np.stack([np.ones(NQ, np.float32), -sqq, 2 * sq[:, 0], 2 * sq[:, 1]])
        rt = np.stack([-sqk, np.ones(N, np.float32), sb[:, 0], sb[:, 1]])
        stcv = np.ascontiguousarray(np.tile(sb.T, (32, 1)).astype(np.float32))
        pgv = np.concatenate([(sq[:, :2] - gq).T, sq[:, 2:].T], axis=0)
        in_maps.append({
            "lt": f32(lt), "rt": f32(rt), "stc": stcv,
            "sqtf": f32(sq.T), "pgv": f32(pgv), **shared,
        })
    return in_maps


def kernel(**inputs):
    in_maps = make_in_maps(**inputs)
    nc = get_nc(None)
    res = run_bass_kernel_spmd(nc, in_maps, list(range(8)))
    out = np.zeros((B, N, D), np.float32)
    for c in range(8):
        b, h = c // 2, c % 2
        out[b, h * NQ : (h + 1) * NQ] = res.results[c]["out"].T
    return out
